# revision 3
# baseline (speedup 1.0000x reference)
"""Exponential smoothing (per-channel EMA over time) on 8 Trainium2 cores.

  s_0 = x_0 ; s_t = a * x_t + (1 - a) * s_{t-1},  a = sigmoid(alpha)  (per channel)

Full shapes: x (16, 4096, 512) f32, alpha (1, 1, 512) f32 -> out (16, 4096, 512).

Design notes (trace-driven):
  * DMA is the hard floor (~360 GB/s per-core bus shared by loads+stores), so
    all HBM I/O is fp16: host preps each core's shard time-major (D, T) fp16
    and upcasts the fp16 result. The 2e-2 global-rel-err budget dwarfs fp16's
    2^-11 rounding; the EMA is a convex combination so input rounding does
    not amplify. Halves DMA bytes vs f32 and kills the on-device transposes
    (channels land on partitions, time on the free axis).
  * The hardware scan (TensorTensorScanArith, vector engine) measured
    ~2 DVE cycles/elem with f32 operands -- the kernel bottleneck. All-fp16
    operands qualify for the DVE 2x/4x packed fast modes (scalar-shaped
    operands like `initial` are exempt and stay f32).
  * The scan runs on RAW x (r_t = w * r_{t-1} + x_t, initial r_0 = x_0 / a,
    so r_t = s_t / a exactly): no pre-scale pass between the DMA load and
    the scan, keeping the scan chain's critical path DMA -> scan. The
    per-channel rescale y = a * r rides the scalar engine after the scan
    (per-partition scale, fp16 out), overlapped with later chains.
  * 8 independent chains (2 batches x 4 channel-chunks of 128), each one
    scan over the full T=4096 free axis: no carry chaining, minimal
    instruction/semaphore count. Loads on the SP HWDGE ring, stores on the
    GpSimd SWDGE ring.
"""

from contextlib import ExitStack

import numpy as np

import concourse.tile as tile
from concourse import bacc, mybir
from concourse.bass_utils import run_bass_kernel_spmd

B, T, D = 16, 4096, 512
NCORES = 8
BL = B // NCORES   # batches per core
P = 128            # partitions
TC = 4096          # time chunk = full T: one scan per chain
ND = D // P        # channel chunks of 128
NTC = T // TC

FP32 = mybir.dt.float32
FP16 = mybir.dt.float16


def build_program(bl: int = BL, t: int = T) -> bacc.Bacc:
    """Build the per-core Bass program (same NEFF for all 8 cores)."""
    ntc = t // TC
    nc = bacc.Bacc(
        "TRN2",
        target_bir_lowering=False,
        debug=False,
        enable_asserts=False,
        num_devices=NCORES,
    )
    x = nc.dram_tensor("xt", (bl, D, t), FP16, kind="ExternalInput").ap()
    alpha = nc.dram_tensor("alpha", (1, 1, D), FP32, kind="ExternalInput").ap()
    y = nc.dram_tensor("yt", (bl, D, t), FP16, kind="ExternalOutput").ap()

    with tile.TileContext(nc) as tc, ExitStack() as ctx:
        const_pool = ctx.enter_context(tc.tile_pool(name="const", bufs=1))
        x_pool = ctx.enter_context(tc.tile_pool(name="x", bufs=5))
        r_pool = ctx.enter_context(tc.tile_pool(name="r", bufs=4))
        y_pool = ctx.enter_context(tc.tile_pool(name="y", bufs=4))
        init_pool = ctx.enter_context(tc.tile_pool(name="init", bufs=2))

        # alpha (1,1,512) -> (128, ND) tile: channel d = j*128 + p
        alpha_sb = const_pool.tile([P, ND], FP32)
        nc.sync.dma_start(alpha_sb[:], alpha.rearrange("o u (j p) -> (o u p) j", p=P))
        a_sb = const_pool.tile([P, ND], FP32)  # a = sigmoid(alpha)
        nc.scalar.activation(a_sb[:], alpha_sb[:], mybir.ActivationFunctionType.Sigmoid)
        w_sb = const_pool.tile([P, ND], FP32)  # w = 1 - a = sigmoid(-alpha)
        nc.scalar.activation(
            w_sb[:], alpha_sb[:], mybir.ActivationFunctionType.Sigmoid, scale=-1.0
        )
        inv_a = const_pool.tile([P, ND], FP32)  # 1/a (f32: reciprocal precision)
        nc.vector.reciprocal(inv_a[:], a_sb[:])

        # Per-channel-chunk decay tiles broadcast along the time axis
        # (scan data0 must be a full, packed [P, TC] operand for the DVE
        # fast mode; fp16 w ulp near 1 is 2^-11 -- fine for the error budget).
        ones = const_pool.tile([P, TC], FP16)
        nc.vector.memset(ones[:], 1.0)
        wbs = []
        for j in range(ND):
            wt = const_pool.tile([P, TC], FP16, tag=f"wb{j}")
            nc.scalar.mul(wt[:], ones[:], w_sb[:, j : j + 1])
            wbs.append(wt)

        # 2*ND = 8 independent scan chains.
        for tci in range(ntc):
            t0 = tci * TC
            for b in range(bl):
                for j in range(ND):
                    xc = x_pool.tile([P, TC], FP16, tag="x")
                    nc.sync.dma_start(xc[:], x[b, j * P : (j + 1) * P, t0 : t0 + TC])
                    # r_0 = x_0 / a: scan col 0 gives w*(x_0/a) + x_0 = x_0/a.
                    init = init_pool.tile([P, 1], FP32, tag="init")
                    nc.scalar.mul(init[:], xc[:, 0:1], inv_a[:, j : j + 1])
                    rc = r_pool.tile([P, TC], FP16, tag="r")
                    nc.vector.tensor_tensor_scan(
                        rc[:],
                        wbs[j][:],
                        xc[:],
                        init[:],
                        mybir.AluOpType.mult,
                        mybir.AluOpType.add,
                    )
                    yc = y_pool.tile([P, TC], FP16, tag="y")
                    nc.scalar.mul(yc[:], rc[:], a_sb[:, j : j + 1])
                    nc.gpsimd.dma_start(y[b, j * P : (j + 1) * P, t0 : t0 + TC], yc[:])

    nc.compile()
    return nc


_prog = None


def _get_prog():
    global _prog
    if _prog is None:
        _prog = build_program()
    return _prog


def make_in_maps(x, alpha):
    """Per-core inputs: time-major fp16 shard of x + replicated alpha."""
    x = np.asarray(x)
    alpha = np.ascontiguousarray(np.asarray(alpha, dtype=np.float32))
    assert x.shape == (B, T, D) and alpha.shape == (1, 1, D)
    xt = np.ascontiguousarray(x.transpose(0, 2, 1)).astype(np.float16)  # (B, D, T)
    return [
        {"xt": np.ascontiguousarray(xt[i * BL : (i + 1) * BL]), "alpha": alpha}
        for i in range(NCORES)
    ]


def gather(results):
    """(NCORES, BL, D, T) fp16 shards -> (B, T, D) f32."""
    yt = np.concatenate([r["yt"] for r in results], axis=0)  # (B, D, T) fp16
    return np.ascontiguousarray(yt.transpose(0, 2, 1)).astype(np.float32)


def kernel(x, alpha):
    res = run_bass_kernel_spmd(
        _get_prog(), make_in_maps(x, alpha), core_ids=list(range(NCORES))
    )
    return gather(res.results)


# revision 4
# speedup vs baseline: 1.4326x; 1.4326x over previous
"""Exponential smoothing (per-channel EMA over time) on 8 Trainium2 cores.

  s_0 = x_0 ; s_t = a * x_t + (1 - a) * s_{t-1},  a = sigmoid(alpha)  (per channel)

Full shapes: x (16, 4096, 512) f32, alpha (1, 1, 512) f32 -> out (16, 4096, 512).

Design (trace-driven):
  * All HBM I/O is fp16, time-major per core: host preps (D, T) fp16 shards
    and upcasts the fp16 result (2e-2 global-rel-err budget vs fp16's 2^-11;
    the EMA is a convex combination, so input rounding does not amplify).
    Halves DMA bytes and puts channels on partitions / time on the free axis
    with no on-device transposes.
  * The hardware scan (TensorTensorScanArith, vector engine) costs ~2.13
    ns/elem/lane regardless of dtype -- the dominant cost. So the kernel
    scans ONLY the odd timesteps (an EMA with decay w^2 over combined
    inputs g_i = w*x_{2i} + x_{2i+1}), halving vector-engine work:
      - combine g: tensor engine, diag(w) @ x_even + I @ x_odd accumulated
        into PSUM (the scan reads data1 straight from PSUM).
      - odd scan: r_{2i+1} = w^2 * r_{2i-1} + g_i on the vector engine.
        It scans r = s/a (initial r_{-1} = x_0/a), which makes x itself the
        scan input -- no pre-scale pass. The scan writes cols 1..NH of an
        [128, NH+1] tile whose col 0 holds the initial, so the shifted
        operand the fill needs is a contiguous slice.
      - even fill: r_{2i} = w * r_{2i-1} + x_{2i}, also on the tensor
        engine: diag(w) @ r_shift + I @ x_even into PSUM.
      - evacuate+scale on the scalar engine: y = a * r for both halves
        (per-partition scale, fp16, stride-2 writes into the output tile).
  * Loads ride the SP HWDGE ring, stores the GpSimd SWDGE ring; both spread
    over all 16 DMA engines (~44 us floor for 16.8 MB/core).
"""

from contextlib import ExitStack

import numpy as np

import concourse.tile as tile
from concourse import bacc, mybir
from concourse.bass_utils import run_bass_kernel_spmd
from concourse.masks import make_identity

B, T, D = 16, 4096, 512
NCORES = 8
BL = B // NCORES   # batches per core
P = 128            # partitions
TC = 2048          # time chunk per pipeline step
NH = TC // 2       # odd (= even) timesteps per chunk
ND = D // P        # channel chunks of 128
MM = 512           # max moving free dim per matmul

FP32 = mybir.dt.float32
FP16 = mybir.dt.float16


def build_program(bl: int = BL, t: int = T) -> bacc.Bacc:
    """Build the per-core Bass program (same NEFF for all 8 cores)."""
    ntc = t // TC
    nc = bacc.Bacc(
        "TRN2",
        target_bir_lowering=False,
        debug=False,
        enable_asserts=False,
        num_devices=NCORES,
    )
    x = nc.dram_tensor("xt", (bl, D, t), FP16, kind="ExternalInput").ap()
    alpha = nc.dram_tensor("alpha", (1, 1, D), FP32, kind="ExternalInput").ap()
    y = nc.dram_tensor("yt", (bl, D, t), FP16, kind="ExternalOutput").ap()

    with tile.TileContext(nc) as tc, ExitStack() as ctx:
        const_pool = ctx.enter_context(tc.tile_pool(name="const", bufs=1))
        x_pool = ctx.enter_context(tc.tile_pool(name="x", bufs=6))
        g_pool = ctx.enter_context(tc.tile_pool(name="g", bufs=2, space="PSUM"))
        re_pool = ctx.enter_context(tc.tile_pool(name="re", bufs=2, space="PSUM"))
        r_pool = ctx.enter_context(tc.tile_pool(name="r", bufs=10))
        y_pool = ctx.enter_context(tc.tile_pool(name="y", bufs=4))

        # alpha (1,1,512) -> (128, ND) tile: channel d = j*128 + p
        alpha_sb = const_pool.tile([P, ND], FP32)
        nc.sync.dma_start(alpha_sb[:], alpha.rearrange("o u (j p) -> (o u p) j", p=P))
        a_sb = const_pool.tile([P, ND], FP32)  # a = sigmoid(alpha)
        nc.scalar.activation(a_sb[:], alpha_sb[:], mybir.ActivationFunctionType.Sigmoid)
        w_sb = const_pool.tile([P, ND], FP32)  # w = 1 - a = sigmoid(-alpha)
        nc.scalar.activation(
            w_sb[:], alpha_sb[:], mybir.ActivationFunctionType.Sigmoid, scale=-1.0
        )
        inv_a = const_pool.tile([P, ND], FP32)  # 1/a
        nc.vector.reciprocal(inv_a[:], a_sb[:])
        w2_sb = const_pool.tile([P, ND], FP32)  # w^2 (odd-subsequence decay)
        nc.vector.tensor_tensor(w2_sb[:], w_sb[:], w_sb[:], mybir.AluOpType.mult)

        # Identity + per-chunk diag(w) stationary matrices for the PE.
        ident = const_pool.tile([P, P], FP16)
        make_identity(nc, ident[:])
        diag_w = []
        for j in range(ND):
            dw = const_pool.tile([P, P], FP16, tag=f"dw{j}")
            nc.vector.tensor_scalar_mul(dw[:], ident[:], w_sb[:, j : j + 1])
            diag_w.append(dw)

        # Scan data0: w^2 broadcast along the time axis (full packed operand).
        ones = const_pool.tile([P, NH], FP16)
        nc.vector.memset(ones[:], 1.0)
        w2b = []
        for j in range(ND):
            wt = const_pool.tile([P, NH], FP16, tag=f"w2b{j}")
            nc.scalar.mul(wt[:], ones[:], w2_sb[:, j : j + 1])
            w2b.append(wt)

        # 8 chains (2 batches x 4 channel chunks), ntc chunks each.
        r_prev = [[None] * ND for _ in range(bl)]
        for tci in range(ntc):
            t0 = tci * TC
            for b in range(bl):
                for j in range(ND):
                    xc = x_pool.tile([P, TC], FP16, tag="x")
                    nc.sync.dma_start(xc[:], x[b, j * P : (j + 1) * P, t0 : t0 + TC])

                    # g_i = w * x_{2i} + x_{2i+1}  (PSUM f32, 2 bank-sized groups)
                    gp = g_pool.tile([P, NH], FP32, tag="g")
                    for h in range(NH // MM):
                        c0, c1 = h * MM, (h + 1) * MM
                        nc.tensor.matmul(
                            gp[:, c0:c1], diag_w[j][:],
                            xc[:, 2 * c0 : 2 * c1 : 2], start=True, stop=False,
                        )
                        nc.tensor.matmul(
                            gp[:, c0:c1], ident[:],
                            xc[:, 2 * c0 + 1 : 2 * c1 : 2], start=False, stop=True,
                        )

                    # ro col 0 = initial r_{-1}; cols 1..NH = scan of odds.
                    # r_{2i+1} = w^2 * r_{2i-1} + g_i, r scans s/a so
                    # initial = x_0 / a (chunk 0) or the previous chunk carry.
                    ro = r_pool.tile([P, NH + 1], FP16, tag="r")
                    if tci == 0:
                        nc.vector.tensor_scalar_mul(
                            ro[:, 0:1], xc[:, 0:1], inv_a[:, j : j + 1]
                        )
                    else:
                        nc.vector.tensor_copy(
                            ro[:, 0:1], r_prev[b][j][:, NH : NH + 1]
                        )
                    nc.vector.tensor_tensor_scan(
                        ro[:, 1 : NH + 1],
                        w2b[j][:],
                        gp[:],
                        ro[:, 0:1],
                        mybir.AluOpType.mult,
                        mybir.AluOpType.add,
                    )

                    # r_{2i} = w * r_{2i-1} + x_{2i}: diag(w) @ ro_shift + I @ x_even
                    rep = re_pool.tile([P, NH], FP32, tag="re")
                    for h in range(NH // MM):
                        c0, c1 = h * MM, (h + 1) * MM
                        nc.tensor.matmul(
                            rep[:, c0:c1], diag_w[j][:],
                            ro[:, c0:c1], start=True, stop=False,
                        )
                        nc.tensor.matmul(
                            rep[:, c0:c1], ident[:],
                            xc[:, 2 * c0 : 2 * c1 : 2], start=False, stop=True,
                        )

                    # y = a * r, interleaved back to natural time order.
                    yc = y_pool.tile([P, TC], FP16, tag="y")
                    nc.scalar.mul(yc[:, 0:TC:2], rep[:], a_sb[:, j : j + 1])
                    nc.scalar.mul(yc[:, 1:TC:2], ro[:, 1 : NH + 1], a_sb[:, j : j + 1])
                    nc.gpsimd.dma_start(y[b, j * P : (j + 1) * P, t0 : t0 + TC], yc[:])
                    r_prev[b][j] = ro

    nc.compile()
    return nc


_prog = None


def _get_prog():
    global _prog
    if _prog is None:
        _prog = build_program()
    return _prog


def make_in_maps(x, alpha):
    """Per-core inputs: time-major fp16 shard of x + replicated alpha."""
    x = np.asarray(x)
    alpha = np.ascontiguousarray(np.asarray(alpha, dtype=np.float32))
    assert x.shape == (B, T, D) and alpha.shape == (1, 1, D)
    xt = np.ascontiguousarray(x.transpose(0, 2, 1)).astype(np.float16)  # (B, D, T)
    return [
        {"xt": np.ascontiguousarray(xt[i * BL : (i + 1) * BL]), "alpha": alpha}
        for i in range(NCORES)
    ]


def gather(results):
    """(NCORES, BL, D, T) fp16 shards -> (B, T, D) f32."""
    yt = np.concatenate([r["yt"] for r in results], axis=0)  # (B, D, T) fp16
    return np.ascontiguousarray(yt.transpose(0, 2, 1)).astype(np.float32)


def kernel(x, alpha):
    res = run_bass_kernel_spmd(
        _get_prog(), make_in_maps(x, alpha), core_ids=list(range(NCORES))
    )
    return gather(res.results)


# revision 7
# speedup vs baseline: 1.5125x; 1.0557x over previous
"""Exponential smoothing (per-channel EMA over time) on 8 Trainium2 cores.

  s_0 = x_0 ; s_t = a * x_t + (1 - a) * s_{t-1},  a = sigmoid(alpha)  (per channel)

Full shapes: x (16, 4096, 512) f32, alpha (1, 1, 512) f32 -> out (16, 4096, 512).

Design (trace-driven):
  * All HBM I/O is fp16, time-major per core: host preps (D, T) fp16 shards
    and upcasts the fp16 result (2e-2 global-rel-err budget vs fp16's 2^-11;
    the EMA is a convex combination, so input rounding does not amplify).
    Halves DMA bytes and puts channels on partitions / time on the free axis
    with no on-device transposes.
  * The hardware scan (TensorTensorScanArith, vector engine) costs ~2.13
    ns/elem/lane regardless of dtype -- the dominant cost. So the kernel
    scans ONLY the odd timesteps (an EMA with decay w^2 over combined
    inputs g_i = w*x_{2i} + x_{2i+1}), halving vector-engine work:
      - combine g: tensor engine, diag(w) @ x_even + I @ x_odd accumulated
        into PSUM (the scan reads data1 straight from PSUM). Issued one
        chunk ahead of the fill so the PE queue never head-of-line blocks
        the next scan behind a fill that waits on the current scan.
      - odd scan: r_{2i+1} = w^2 * r_{2i-1} + g_i on the vector engine.
        It scans r = s/a (initial r_{-1} = x_0/a), which makes x itself the
        scan input -- no pre-scale pass. The scan writes cols 1..NH of an
        [128, NH+1] tile whose col 0 holds the initial, so the shifted
        operand the fill needs is a contiguous slice.
      - even fill: r_{2i} = w * r_{2i-1} + x_{2i}, also on the tensor
        engine. Matmuls are grouped per stationary matrix (both diag(w)
        halves, then both identity halves) to halve LDWEIGHTS switches.
      - evacuate+scale y = a * r: evens (PSUM source) on the scalar engine,
        odds (SBUF source) alternating scalar/gpsimd. y is written in
        blocked half layout [evens | odds] per chunk -- contiguous writes;
        the host de-interleaves (pure layout transform).
  * Loads ride the SP HWDGE ring, stores the GpSimd SWDGE ring; both spread
    over all 16 DMA engines (~44 us floor for 16.8 MB/core).
"""

from contextlib import ExitStack

import numpy as np

import concourse.tile as tile
from concourse import bacc, mybir
from concourse.bass_utils import run_bass_kernel_spmd
from concourse.masks import make_identity

B, T, D = 16, 4096, 512
NCORES = 8
BL = B // NCORES   # batches per core
P = 128            # partitions
TC = 2048          # time chunk per pipeline step
NH = TC // 2       # odd (= even) timesteps per chunk
ND = D // P        # channel chunks of 128
MM = 512           # max moving free dim per matmul
NTC = T // TC

FP32 = mybir.dt.float32
FP16 = mybir.dt.float16


def build_program(bl: int = BL, t: int = T) -> bacc.Bacc:
    """Build the per-core Bass program (same NEFF for all 8 cores)."""
    ntc = t // TC
    nc = bacc.Bacc(
        "TRN2",
        target_bir_lowering=False,
        debug=False,
        enable_asserts=False,
        num_devices=NCORES,
    )
    x = nc.dram_tensor("xt", (bl, D, t), FP16, kind="ExternalInput").ap()
    alpha = nc.dram_tensor("alpha", (1, 1, D), FP32, kind="ExternalInput").ap()
    y = nc.dram_tensor("yt", (bl, D, t), FP16, kind="ExternalOutput").ap()

    with tile.TileContext(nc) as tc, ExitStack() as ctx:
        const_pool = ctx.enter_context(tc.tile_pool(name="const", bufs=1))
        x_pool = ctx.enter_context(tc.tile_pool(name="x", bufs=6))
        g_pool = ctx.enter_context(tc.tile_pool(name="g", bufs=2, space="PSUM"))
        re_pool = ctx.enter_context(tc.tile_pool(name="re", bufs=2, space="PSUM"))
        r_pool = ctx.enter_context(tc.tile_pool(name="r", bufs=10))
        y_pool = ctx.enter_context(tc.tile_pool(name="y", bufs=4))

        # Identity first: gpsimd builds it while alpha DMA + sigmoids run.
        ident = const_pool.tile([P, P], FP16)
        make_identity(nc, ident[:])

        # alpha (1,1,512) -> (128, ND) tile: channel d = j*128 + p
        alpha_sb = const_pool.tile([P, ND], FP32)
        nc.sync.dma_start(alpha_sb[:], alpha.rearrange("o u (j p) -> (o u p) j", p=P))
        a_sb = const_pool.tile([P, ND], FP32)  # a = sigmoid(alpha)
        nc.scalar.activation(a_sb[:], alpha_sb[:], mybir.ActivationFunctionType.Sigmoid)
        w_sb = const_pool.tile([P, ND], FP32)  # w = 1 - a = sigmoid(-alpha)
        nc.scalar.activation(
            w_sb[:], alpha_sb[:], mybir.ActivationFunctionType.Sigmoid, scale=-1.0
        )
        inv_a = const_pool.tile([P, ND], FP32)  # 1/a
        nc.vector.reciprocal(inv_a[:], a_sb[:])
        w2_sb = const_pool.tile([P, ND], FP32)  # w^2 (odd-subsequence decay)
        nc.vector.tensor_tensor(w2_sb[:], w_sb[:], w_sb[:], mybir.AluOpType.mult)

        diag_w = []
        for j in range(ND):
            dw = const_pool.tile([P, P], FP16, tag=f"dw{j}")
            nc.vector.tensor_scalar_mul(dw[:], ident[:], w_sb[:, j : j + 1])
            diag_w.append(dw)

        # Scan data0: w^2 broadcast along the time axis (full packed operand).
        ones = const_pool.tile([P, NH], FP16)
        nc.vector.memset(ones[:], 1.0)
        w2b = []
        for j in range(ND):
            wt = const_pool.tile([P, NH], FP16, tag=f"w2b{j}")
            nc.scalar.mul(wt[:], ones[:], w2_sb[:, j : j + 1])
            w2b.append(wt)

        # 8 chains (2 batches x 4 channel chunks), ntc chunks each, chunk
        # order: all chunk-0s, then all chunk-1s (carries ready early).
        chunks = [
            (tci, b, j) for tci in range(ntc) for b in range(bl) for j in range(ND)
        ]
        n = len(chunks)
        xcs, gps = [None] * n, [None] * n
        r_prev = [[None] * ND for _ in range(bl)]

        def load_and_combine(c):
            tci, b, j = chunks[c]
            t0 = tci * TC
            xc = x_pool.tile([P, TC], FP16, tag="x")
            nc.sync.dma_start(xc[:], x[b, j * P : (j + 1) * P, t0 : t0 + TC])
            # g_i = w * x_{2i} + x_{2i+1}: per-stationary batched matmuls.
            gp = g_pool.tile([P, NH], FP32, tag="g")
            for h in range(NH // MM):
                c0, c1 = h * MM, (h + 1) * MM
                nc.tensor.matmul(
                    gp[:, c0:c1], diag_w[j][:],
                    xc[:, 2 * c0 : 2 * c1 : 2], start=True, stop=False,
                )
            for h in range(NH // MM):
                c0, c1 = h * MM, (h + 1) * MM
                nc.tensor.matmul(
                    gp[:, c0:c1], ident[:],
                    xc[:, 2 * c0 + 1 : 2 * c1 : 2], start=False, stop=True,
                )
            xcs[c], gps[c] = xc, gp

        load_and_combine(0)
        load_and_combine(1)
        for c in range(n):
            tci, b, j = chunks[c]
            t0 = tci * TC
            xc, gp = xcs[c], gps[c]

            # ro col 0 = initial r_{-1}; cols 1..NH = scan of odds.
            ro = r_pool.tile([P, NH + 1], FP16, tag="r")
            if tci == 0:
                nc.vector.tensor_scalar_mul(
                    ro[:, 0:1], xc[:, 0:1], inv_a[:, j : j + 1]
                )
            else:
                nc.vector.tensor_copy(ro[:, 0:1], r_prev[b][j][:, NH : NH + 1])
            nc.vector.tensor_tensor_scan(
                ro[:, 1 : NH + 1],
                w2b[j][:],
                gp[:],
                ro[:, 0:1],
                mybir.AluOpType.mult,
                mybir.AluOpType.add,
            )
            r_prev[b][j] = ro

            # Next chunk's combine goes on the PE queue BEFORE this fill.
            if c + 2 < n:
                load_and_combine(c + 2)

            # r_{2i} = w * r_{2i-1} + x_{2i}: diag(w) @ ro_shift + I @ x_even
            rep = re_pool.tile([P, NH], FP32, tag="re")
            for h in range(NH // MM):
                c0, c1 = h * MM, (h + 1) * MM
                nc.tensor.matmul(
                    rep[:, c0:c1], diag_w[j][:], ro[:, c0:c1], start=True, stop=False
                )
            for h in range(NH // MM):
                c0, c1 = h * MM, (h + 1) * MM
                nc.tensor.matmul(
                    rep[:, c0:c1], ident[:],
                    xc[:, 2 * c0 : 2 * c1 : 2], start=False, stop=True,
                )

            # y = a * r, blocked [evens | odds]; host de-interleaves.
            yc = y_pool.tile([P, TC], FP16, tag="y")
            nc.scalar.mul(yc[:, 0:NH], rep[:], a_sb[:, j : j + 1])
            nc.scalar.mul(yc[:, NH:TC], ro[:, 1 : NH + 1], a_sb[:, j : j + 1])
            nc.gpsimd.dma_start(y[b, j * P : (j + 1) * P, t0 : t0 + TC], yc[:])

    nc.compile()
    return nc


_prog = None


def _get_prog():
    global _prog
    if _prog is None:
        _prog = build_program()
    return _prog


def make_in_maps(x, alpha):
    """Per-core inputs: time-major fp16 shard of x + replicated alpha."""
    x = np.asarray(x)
    alpha = np.ascontiguousarray(np.asarray(alpha, dtype=np.float32))
    assert x.shape == (B, T, D) and alpha.shape == (1, 1, D)
    xt = np.ascontiguousarray(x.transpose(0, 2, 1)).astype(np.float16)  # (B, D, T)
    return [
        {"xt": np.ascontiguousarray(xt[i * BL : (i + 1) * BL]), "alpha": alpha}
        for i in range(NCORES)
    ]


def gather(results):
    """(NCORES, BL, D, T) fp16 blocked shards -> (B, T, D) f32.

    Per TC chunk the device wrote [NH evens | NH odds]; interleave back.
    """
    yt = np.concatenate([r["yt"] for r in results], axis=0)  # (B, D, T) blocked
    blk = yt.reshape(B, D, NTC, 2, NH)  # [..., 0, :] evens, [..., 1, :] odds
    nat = blk.transpose(0, 1, 2, 4, 3).reshape(B, D, T)  # interleave
    return np.ascontiguousarray(nat.transpose(0, 2, 1)).astype(np.float32)


def kernel(x, alpha):
    res = run_bass_kernel_spmd(
        _get_prog(), make_in_maps(x, alpha), core_ids=list(range(NCORES))
    )
    return gather(res.results)


# revision 8
# speedup vs baseline: 1.5206x; 1.0054x over previous
"""Exponential smoothing (per-channel EMA over time) on 8 Trainium2 cores.

  s_0 = x_0 ; s_t = a * x_t + (1 - a) * s_{t-1},  a = sigmoid(alpha)  (per channel)

Full shapes: x (16, 4096, 512) f32, alpha (1, 1, 512) f32 -> out (16, 4096, 512).

Design (trace-driven):
  * All HBM I/O is fp16, time-major per core: host preps (D, T) fp16 shards
    and upcasts the fp16 result (2e-2 global-rel-err budget vs fp16's 2^-11;
    the EMA is a convex combination, so input rounding does not amplify).
    Halves DMA bytes and puts channels on partitions / time on the free axis
    with no on-device transposes. The tiny per-channel parameter transforms
    (a = sigmoid(alpha), w, w^2, 1/a -- 512 elements) are precomputed on the
    host and shipped as one contiguous (128, 16) f32 tile.
  * The hardware scan (TensorTensorScanArith, vector engine) costs ~2.13
    ns/elem/lane regardless of dtype -- the dominant cost. So the kernel
    scans ONLY the odd timesteps (an EMA with decay w^2 over combined
    inputs g_i = w*x_{2i} + x_{2i+1}), halving vector-engine work:
      - combine g: tensor engine, diag(w) @ x_even + I @ x_odd accumulated
        into PSUM (the scan reads data1 straight from PSUM). Combines are
        issued one chunk ahead of the fill, and g triple-buffers, so the PE
        stream never stalls (the PE clock only ramps 1.2 -> 2.4 GHz after
        ~3 us of continuous execution).
      - odd scan: r_{2i+1} = w^2 * r_{2i-1} + g_i on the vector engine.
        It scans r = s/a (initial r_{-1} = x_0/a), which makes x itself the
        scan input -- no pre-scale pass. The scan writes cols 1..NH of an
        [128, NH+1] tile whose col 0 holds the initial, so the shifted
        operand the fill needs is a contiguous slice.
      - even fill: r_{2i} = w * r_{2i-1} + x_{2i}, also on the tensor
        engine. Matmuls are grouped per stationary matrix (both diag(w)
        halves, then both identity halves) to halve LDWEIGHTS switches.
      - evacuate+scale y = a * r: evens (PSUM source) on the scalar engine;
        odds (SBUF source) alternate scalar engine / vector engine (the
        fp16-packed DVE fast mode makes them cheap; gpsimd elementwise ops
        crash the Q7 handler -- do not use). y is written in blocked half
        layout [evens | odds] per chunk; the host de-interleaves.
  * Loads ride the SP HWDGE ring, stores the GpSimd SWDGE ring; both spread
    over all 16 DMA engines (~44 us floor for 16.8 MB/core).
"""

from contextlib import ExitStack

import numpy as np

import concourse.tile as tile
from concourse import bacc, mybir
from concourse.bass_utils import run_bass_kernel_spmd
from concourse.masks import make_identity

B, T, D = 16, 4096, 512
NCORES = 8
BL = B // NCORES   # batches per core
P = 128            # partitions
TC = 2048          # time chunk per pipeline step
NH = TC // 2       # odd (= even) timesteps per chunk
ND = D // P        # channel chunks of 128
MM = 512           # max moving free dim per matmul
NTC = T // TC

FP32 = mybir.dt.float32
FP16 = mybir.dt.float16


def build_program(bl: int = BL, t: int = T) -> bacc.Bacc:
    """Build the per-core Bass program (same NEFF for all 8 cores)."""
    ntc = t // TC
    nc = bacc.Bacc(
        "TRN2",
        target_bir_lowering=False,
        debug=False,
        enable_asserts=False,
        num_devices=NCORES,
    )
    x = nc.dram_tensor("xt", (bl, D, t), FP16, kind="ExternalInput").ap()
    # Host-precomputed per-channel coefficients, partition-major:
    # col q*ND + j = quantity q for channel chunk j (q: 0=a, 1=w, 2=w^2, 3=1/a)
    coef = nc.dram_tensor("coef", (P, 4 * ND), FP32, kind="ExternalInput").ap()
    y = nc.dram_tensor("yt", (bl, D, t), FP16, kind="ExternalOutput").ap()

    with tile.TileContext(nc) as tc, ExitStack() as ctx:
        const_pool = ctx.enter_context(tc.tile_pool(name="const", bufs=1))
        x_pool = ctx.enter_context(tc.tile_pool(name="x", bufs=6))
        g_pool = ctx.enter_context(tc.tile_pool(name="g", bufs=3, space="PSUM"))
        re_pool = ctx.enter_context(tc.tile_pool(name="re", bufs=1, space="PSUM"))
        r_pool = ctx.enter_context(tc.tile_pool(name="r", bufs=10))
        y_pool = ctx.enter_context(tc.tile_pool(name="y", bufs=4))

        # Identity first: gpsimd builds it while the coef DMA runs.
        ident = const_pool.tile([P, P], FP16)
        make_identity(nc, ident[:])

        coef_sb = const_pool.tile([P, 4 * ND], FP32)
        nc.sync.dma_start(coef_sb[:], coef[:, :])
        a_sb = coef_sb[:, 0 * ND : 1 * ND]
        w_sb = coef_sb[:, 1 * ND : 2 * ND]
        w2_sb = coef_sb[:, 2 * ND : 3 * ND]
        inv_a = coef_sb[:, 3 * ND : 4 * ND]

        diag_w = []
        for j in range(ND):
            dw = const_pool.tile([P, P], FP16, tag=f"dw{j}")
            nc.vector.tensor_scalar_mul(dw[:], ident[:], w_sb[:, j : j + 1])
            diag_w.append(dw)

        # Scan data0: w^2 broadcast along the time axis (full packed operand).
        ones = const_pool.tile([P, NH], FP16)
        nc.vector.memset(ones[:], 1.0)
        w2b = []
        for j in range(ND):
            wt = const_pool.tile([P, NH], FP16, tag=f"w2b{j}")
            nc.scalar.mul(wt[:], ones[:], w2_sb[:, j : j + 1])
            w2b.append(wt)

        # 8 chains (2 batches x 4 channel chunks), ntc chunks each, chunk
        # order: all chunk-0s, then all chunk-1s (carries ready early).
        chunks = [
            (tci, b, j) for tci in range(ntc) for b in range(bl) for j in range(ND)
        ]
        n = len(chunks)
        xcs, gps = [None] * n, [None] * n
        r_prev = [[None] * ND for _ in range(bl)]

        def load_and_combine(c):
            tci, b, j = chunks[c]
            t0 = tci * TC
            xc = x_pool.tile([P, TC], FP16, tag="x")
            nc.sync.dma_start(xc[:], x[b, j * P : (j + 1) * P, t0 : t0 + TC])
            # g_i = w * x_{2i} + x_{2i+1}: per-stationary batched matmuls.
            gp = g_pool.tile([P, NH], FP32, tag="g")
            for h in range(NH // MM):
                c0, c1 = h * MM, (h + 1) * MM
                nc.tensor.matmul(
                    gp[:, c0:c1], diag_w[j][:],
                    xc[:, 2 * c0 : 2 * c1 : 2], start=True, stop=False,
                )
            for h in range(NH // MM):
                c0, c1 = h * MM, (h + 1) * MM
                nc.tensor.matmul(
                    gp[:, c0:c1], ident[:],
                    xc[:, 2 * c0 + 1 : 2 * c1 : 2], start=False, stop=True,
                )
            xcs[c], gps[c] = xc, gp

        load_and_combine(0)
        load_and_combine(1)
        for c in range(n):
            tci, b, j = chunks[c]
            t0 = tci * TC
            xc, gp = xcs[c], gps[c]

            # ro col 0 = initial r_{-1}; cols 1..NH = scan of odds.
            ro = r_pool.tile([P, NH + 1], FP16, tag="r")
            if tci == 0:
                nc.vector.tensor_scalar_mul(
                    ro[:, 0:1], xc[:, 0:1], inv_a[:, j : j + 1]
                )
            else:
                nc.vector.tensor_copy(ro[:, 0:1], r_prev[b][j][:, NH : NH + 1])
            nc.vector.tensor_tensor_scan(
                ro[:, 1 : NH + 1],
                w2b[j][:],
                gp[:],
                ro[:, 0:1],
                mybir.AluOpType.mult,
                mybir.AluOpType.add,
            )
            r_prev[b][j] = ro

            # Next chunk's combine goes on the PE queue BEFORE this fill.
            if c + 2 < n:
                load_and_combine(c + 2)

            # r_{2i} = w * r_{2i-1} + x_{2i}: diag(w) @ ro_shift + I @ x_even
            rep = re_pool.tile([P, NH], FP32, tag="re")
            for h in range(NH // MM):
                c0, c1 = h * MM, (h + 1) * MM
                nc.tensor.matmul(
                    rep[:, c0:c1], diag_w[j][:], ro[:, c0:c1], start=True, stop=False
                )
            for h in range(NH // MM):
                c0, c1 = h * MM, (h + 1) * MM
                nc.tensor.matmul(
                    rep[:, c0:c1], ident[:],
                    xc[:, 2 * c0 : 2 * c1 : 2], start=False, stop=True,
                )

            # y = a * r, blocked [evens | odds]; host de-interleaves.
            yc = y_pool.tile([P, TC], FP16, tag="y")
            nc.scalar.mul(yc[:, 0:NH], rep[:], a_sb[:, j : j + 1])
            if b == 0:
                nc.scalar.mul(yc[:, NH:TC], ro[:, 1 : NH + 1], a_sb[:, j : j + 1])
            else:
                nc.vector.tensor_scalar_mul(
                    yc[:, NH:TC], ro[:, 1 : NH + 1], a_sb[:, j : j + 1]
                )
            nc.gpsimd.dma_start(y[b, j * P : (j + 1) * P, t0 : t0 + TC], yc[:])

    nc.compile()
    return nc


_prog = None


def _get_prog():
    global _prog
    if _prog is None:
        _prog = build_program()
    return _prog


def make_coef(alpha):
    """Precompute per-channel (a, w, w^2, 1/a) packed as (128, 4*ND) f32."""
    al = np.asarray(alpha, dtype=np.float64).reshape(D)
    a = 1.0 / (1.0 + np.exp(-al))
    w = 1.0 - a
    quants = [a, w, w * w, 1.0 / a]
    out = np.empty((P, 4 * ND), dtype=np.float32)
    for q, v in enumerate(quants):
        # channel d = j*128 + p -> column q*ND + j, row p
        out[:, q * ND : (q + 1) * ND] = v.reshape(ND, P).T
    return out


def make_in_maps(x, alpha):
    """Per-core inputs: time-major fp16 shard of x + replicated coef."""
    x = np.asarray(x)
    alpha = np.asarray(alpha)
    assert x.shape == (B, T, D) and alpha.shape == (1, 1, D)
    coef = make_coef(alpha)
    xt = np.ascontiguousarray(x.transpose(0, 2, 1)).astype(np.float16)  # (B, D, T)
    return [
        {"xt": np.ascontiguousarray(xt[i * BL : (i + 1) * BL]), "coef": coef}
        for i in range(NCORES)
    ]


def gather(results):
    """(NCORES, BL, D, T) fp16 blocked shards -> (B, T, D) f32.

    Per TC chunk the device wrote [NH evens | NH odds]; interleave back.
    """
    yt = np.concatenate([r["yt"] for r in results], axis=0)  # (B, D, T) blocked
    blk = yt.reshape(B, D, NTC, 2, NH)  # [..., 0, :] evens, [..., 1, :] odds
    nat = blk.transpose(0, 1, 2, 4, 3).reshape(B, D, T)  # interleave
    return np.ascontiguousarray(nat.transpose(0, 2, 1)).astype(np.float32)


def kernel(x, alpha):
    res = run_bass_kernel_spmd(
        _get_prog(), make_in_maps(x, alpha), core_ids=list(range(NCORES))
    )
    return gather(res.results)
